# revision 24
# baseline (speedup 1.0000x reference)
"""Trainium2 Bass kernel for nn_GaussianEmbedding.

Y[b,n,c] = h[0,c] + sum_{i=1..8} h[i,c] * diag(A^i)[b,n]

Math: with full powers A^1..A^4 available, every diag(A^i) for i=2..8 is a
"rows of A^p dot cols of A^q" (p+q=i) — computed on the tensor engine as 16
diagonal [128,128] blocks, 1/16 the cost of a full matmul.

Precision split: the output Frobenius norm is dominated by the i=8 term
(|diag(A^i)| ~ 2048^{(i-1)/2}, so each lower power is diluted ~45x).  Only
the D8 chain (A2 = A@A, A4 = A2@A2, D8 = diag(A4*A4)) needs bf16; A3 and the
D5/D6/D7 diag blocks run in fp8e4 with the DoubleRow perf mode (two
contraction planes per pass = 2x PE throughput).  Power-of-2 scales keep fp8
operands in range (max |x| ~ 240) and are folded into the host-side hh rows.

Per-core schedule (one [2048,2048] A per NeuronCore, 8 cores data-parallel):
  P1: A2   = A@A     bf16 (lhsT = A^T DMA-transpose chunks; rhs = A slabs)
      + evict A2 -> DRAM bf16 + resident fp8 (2^-1); A^T chunks -> fp8 AT8
      + D1 (vector), D2 (PSUM diag)
  P2: A4   = A2@A2   bf16 (lhsT = A2^T transpose chunks -> also fp8 -> DRAM;
      rhs = A2 slabs reloaded) + evict A4 -> DRAM bf16 + resident fp8 (2^-12)
      + D4 (PSUM diag)
  P3: A3^T = A2^T@A^T  fp8 DoubleRow (lhsT = A28 col-slices; rhs = AT8)
      evict -> resident fp8 A3T8 (2^-6 extra); D3 (PSUM diag, x2)
  P4: D5 = diag(A3*A2), D6 = diag(A2*A4), D7 = diag(A3*A4) fp8 DoubleRow;
      D8 = diag(A4*A4) bf16 (lhsT = A4^T transpose chunks; rhs = A4 cols)
      + P5 folded in two blocks behind
  P5: Y = D'^T @ hh  (PE transpose of diag tile + K=9 fp32 matmul)
"""

import numpy as np

NUM_TERMS = 8
C = 64
BATCH = 8
N = 2048
P = 128
N_CORES = 8

# fp8 storage scales (powers of 2); folded into hh on the host.
S_A2 = 0.5        # A2 stored as A2 * 2^-1   (sigma 45 -> 22.5)
S_A4 = 2.0 ** -12  # A4 stored as A4 * 2^-12  (sigma 92681 -> 22.6)
S_A3 = 2.0 ** -6   # extra scale on A3^T evict; PSUM already carries S_A2

_RUNNER = None
_NC = None


def _bf16(x: np.ndarray):
    """Fast float32 -> bfloat16 with round-to-nearest-even (vectorized)."""
    import ml_dtypes

    u = np.ascontiguousarray(x, np.float32).view(np.uint32)
    r = (u >> np.uint32(16)) & np.uint32(1)
    out = ((u + np.uint32(0x7FFF) + r) >> np.uint32(16)).astype(np.uint16)
    return out.view(ml_dtypes.bfloat16)


def _build(n: int = N, reps: int = 1):
    import concourse.tile as tile
    from concourse import bacc, mybir
    from concourse.masks import make_identity

    f32 = mybir.dt.float32
    bf16 = mybir.dt.bfloat16
    f8 = mybir.dt.float8e4
    DR = mybir.MatmulPerfMode.DoubleRow
    mult = mybir.AluOpType.mult
    add = mybir.AluOpType.add
    AX = mybir.AxisListType.X

    ko = n // P      # contraction chunks / row-slabs
    nb = n // P      # diagonal blocks
    nw = n // 512    # 512-wide output chunks per row-block

    nc = bacc.Bacc("TRN2", target_bir_lowering=False, num_devices=N_CORES)

    A_in = nc.declare_dram_parameter("A", [n, n], bf16, isOutput=False)
    hh_in = nc.declare_dram_parameter("hh", [16, C], f32, isOutput=False)
    Y_out = nc.declare_dram_parameter("Y", [n, C], f32, isOutput=True)

    A2_d = nc.dram_tensor("A2_d", [n, n], bf16)
    A4_d = nc.dram_tensor("A4_d", [n, n], bf16)
    # A2^T fp8, stored as row-blocks of A2^T (block m rows = A2^T[mP:(m+1)P,:])
    A2T8_d = nc.dram_tensor("A2T8_d", [n, n], f8)

    with tile.TileContext(nc) as tc:
        with (
            tc.tile_pool(name="res8", bufs=1) as res8,
            tc.tile_pool(name="small", bufs=1) as small,
            tc.tile_pool(name="chunks", bufs=2) as chunks,
            tc.tile_pool(name="ev", bufs=4) as evp,
            tc.tile_pool(name="mmps", bufs=4, space="PSUM") as psp,
            tc.tile_pool(name="dgps", bufs=2, space="PSUM") as dpsp,
        ):
            # fp8 residents, alive across phases:
            #   AT8[p,kk,c]  = A^T[kk*P+p, c]           (P3 rhs)
            #   A28[p,kk,c]  = 2^-1  A2[kk*P+p, c]      (P3 lhsT slices, D5 rhs)
            #   A48[p,kk,c]  = 2^-12 A4[kk*P+p, c]      (D6/D7 rhs)
            AT8 = res8.tile([P, ko, n], f8, tag="AT8")
            A28 = res8.tile([P, ko, n], f8, tag="A28")
            A48 = res8.tile([P, ko, n], f8, tag="A48")

            identf = small.tile([P, P], f32)
            identb = small.tile([P, P], bf16)
            make_identity(nc, identf)
            make_identity(nc, identb)
            hh_sb = small.tile([16, C], f32)
            nc.sync.dma_start(hh_sb, hh_in[:, :])
            # Dall[:, blk, t]: t=0..7 -> (scaled) diag(A^(t+1)); t=8 -> 1.0
            Dall = small.tile([P, nb, 16], f32)
            nc.any.memset(Dall[:, :, 8:9], 1.0)

            def psum_diag(ps, m, nci, tslot):
                """Extract the diag [P] of out-row-block m from PSUM window."""
                if nci != (m * P) // 512:
                    return
                off = (m * P) % 512
                dt = evp.tile([P, P], f32, tag="dtmp")
                nc.vector.tensor_tensor(dt, ps[:, off : off + P], identf, mult)
                nc.vector.tensor_reduce(Dall[:, m, tslot : tslot + 1], dt, AX, add)

            def diag_f8(tslot, b, lhsT_sb, rhs_sb):
                """Dall[:, b, tslot] via fp8 DoubleRow diag block b."""
                dps = dpsp.tile([P, P], f32, tag="dg")
                for j in range(ko // 2):
                    nc.tensor.matmul(
                        dps,
                        lhsT_sb[:, 2 * j : 2 * j + 2, :],
                        rhs_sb[:, 2 * j : 2 * j + 2, :],
                        start=(j == 0),
                        stop=(j == ko // 2 - 1),
                        perf_mode=DR,
                    )
                dt = evp.tile([P, P], f32, tag="dtmp")
                nc.vector.tensor_tensor(dt, dps, identf, mult)
                nc.vector.tensor_reduce(Dall[:, b, tslot : tslot + 1], dt, AX, add)

            for _rep in range(reps):
                with tc.tile_pool(name="slabs", bufs=1) as slabs:
                    # A (P1) then A2 (P2) row-slabs, rhs for the bf16 matmuls.
                    bigA_s = [
                        slabs.tile([P, n], bf16, tag=f"bigA{i}", name=f"bigA{i}")
                        for i in range(ko)
                    ]

                    # ---- P0: prefetch first lhsT transpose chunks, then bulk A ----
                    atm_pre = []
                    for m in range(2):
                        atm = chunks.tile(
                            [P, ko, P], bf16, tag="tch", bufs=4, name=f"atm_pre{m}"
                        )
                        nc.sync.dma_start_transpose(atm, A_in[m * P : (m + 1) * P, :])
                        atm_pre.append(atm)
                    for mb in range(ko):
                        nc.sync.dma_start(bigA_s[mb], A_in[mb * P : (mb + 1) * P, :])

                    # ---- P1: A2 = A@A bf16; AT8/A28 fp8 side-products; D1, D2 ----
                    for m in range(nb):
                        if m < 2:
                            atm = atm_pre[m]
                        else:
                            atm = chunks.tile([P, ko, P], bf16, tag="tch", bufs=4)
                            nc.sync.dma_start_transpose(
                                atm, A_in[m * P : (m + 1) * P, :]
                            )
                        nc.any.tensor_copy(out=AT8[:, :, m * P : (m + 1) * P], in_=atm)
                        for nci in range(nw):
                            ps = psp.tile([P, 512], f32, tag="mm")
                            for kk in range(ko):
                                nc.tensor.matmul(
                                    ps,
                                    atm[:, kk, :],
                                    bigA_s[kk][:, nci * 512 : (nci + 1) * 512],
                                    start=(kk == 0),
                                    stop=(kk == ko - 1),
                                )
                            psum_diag(ps, m, nci, 1)  # D2
                            ev = evp.tile([P, 512], bf16, tag="ev")
                            nc.any.tensor_copy(out=ev, in_=ps)
                            nc.sync.dma_start(
                                A2_d[m * P : (m + 1) * P, nci * 512 : (nci + 1) * 512],
                                ev,
                            )
                            nc.any.tensor_scalar_mul(
                                A28[:, m, nci * 512 : (nci + 1) * 512], ps, S_A2
                            )
                        dt1 = evp.tile([P, P], f32, tag="dtmp")
                        nc.vector.tensor_tensor(
                            dt1, bigA_s[m][:, m * P : (m + 1) * P], identb, mult
                        )
                        nc.vector.tensor_reduce(Dall[:, m, 0:1], dt1, AX, add)  # D1

                    # ---- P2: A4 = A2@A2 bf16 (lhsT = A2^T chunks, also -> fp8
                    # DRAM for D6); rhs = A2 slabs reloaded; D4 from PSUM ----
                    for mb in range(ko):
                        nc.sync.dma_start(bigA_s[mb], A2_d[mb * P : (mb + 1) * P, :])
                    for m in range(nb):
                        atm2 = chunks.tile([P, ko, P], bf16, tag="tch", bufs=4)
                        nc.sync.dma_start_transpose(atm2, A2_d[m * P : (m + 1) * P, :])
                        a2t8 = chunks.tile([P, ko, P], f8, tag="t8", bufs=3)
                        nc.any.tensor_scalar_mul(a2t8, atm2, S_A2)
                        nc.sync.dma_start(A2T8_d[m * P : (m + 1) * P, :], a2t8)
                        for nci in range(nw):
                            ps = psp.tile([P, 512], f32, tag="mm")
                            for kk in range(ko):
                                nc.tensor.matmul(
                                    ps,
                                    atm2[:, kk, :],
                                    bigA_s[kk][:, nci * 512 : (nci + 1) * 512],
                                    start=(kk == 0),
                                    stop=(kk == ko - 1),
                                )
                            psum_diag(ps, m, nci, 3)  # D4
                            ev = evp.tile([P, 512], bf16, tag="ev")
                            nc.any.tensor_copy(out=ev, in_=ps)
                            nc.sync.dma_start(
                                A4_d[m * P : (m + 1) * P, nci * 512 : (nci + 1) * 512],
                                ev,
                            )
                            nc.any.tensor_scalar_mul(
                                A48[:, m, nci * 512 : (nci + 1) * 512], ps, S_A4
                            )

                # slabs pool released: its 8MB holds A3T8 + P4 stream rings.
                with tc.tile_pool(name="ph34", bufs=1) as p34:
                    A3T8 = p34.tile([P, ko, n], f8, tag="A3T8")

                    # ---- P3: A3^T = A2^T @ A^T, fp8 DoubleRow.  Row-block m of
                    # A3^T: lhsT = A28 col-slices (= A2^T rows), rhs = AT8. ----
                    for m in range(nb):
                        for nci in range(nw):
                            ps = psp.tile([P, 512], f32, tag="mm")
                            for j in range(ko // 2):
                                nc.tensor.matmul(
                                    ps,
                                    A28[:, 2 * j : 2 * j + 2, m * P : (m + 1) * P],
                                    AT8[:, 2 * j : 2 * j + 2, nci * 512 : (nci + 1) * 512],
                                    start=(j == 0),
                                    stop=(j == ko // 2 - 1),
                                    perf_mode=DR,
                                )
                            psum_diag(ps, m, nci, 2)  # D3 (x 2^-1)
                            nc.any.tensor_scalar_mul(
                                A3T8[:, m, nci * 512 : (nci + 1) * 512], ps, S_A3
                            )

                    # ---- P5 (interleaved): one Dall block -> Y block ----
                    DT = small.tile([16, nb, P], f32)
                    y_sb = small.tile([P, nb, C], f32)

                    def p5_t(no):
                        tp = dpsp.tile([16, P], f32, tag="tp", bufs=1)
                        nc.tensor.transpose(tp[:9, :], Dall[:, no, 0:9], identf)
                        nc.any.tensor_copy(out=DT[:9, no, :], in_=tp[:9, :])

                    def p5_y(no):
                        yp = dpsp.tile([P, C], f32, tag="yp", bufs=1)
                        nc.tensor.matmul(
                            yp, DT[:9, no, :], hh_sb[:9, :], start=True, stop=True
                        )
                        nc.any.tensor_copy(out=y_sb[:, no, :], in_=yp)

                    # ---- P4: D5..D8 diag blocks (+ P5 folded two behind) ----
                    for b in range(nb):
                        cs = slice(b * P, (b + 1) * P)
                        # Issue all three stream DMAs first so the queue runs
                        # ahead of the PE by the full ring depth.
                        a2t8b = chunks.tile([P, ko, P], f8, tag="t8", bufs=3)
                        nc.sync.dma_start(a2t8b, A2T8_d[b * P : (b + 1) * P, :])
                        a4tb = p34.tile([P, ko, P], bf16, tag="tch4", bufs=6)
                        nc.sync.dma_start_transpose(a4tb, A4_d[b * P : (b + 1) * P, :])
                        a4cb = p34.tile([P, ko, P], bf16, tag="tc2", bufs=3)
                        nc.sync.dma_start(
                            a4cb,
                            A4_d.ap()[:, b * P : (b + 1) * P].rearrange(
                                "(kk p) c -> p kk c", p=P
                            ),
                        )
                        # D5 = diag(A3*A2): lhsT = A3^T slices, rhs = A2 slices
                        diag_f8(4, b, A3T8[:, :, cs], A28[:, :, cs])
                        # D6 = diag(A2*A4): lhsT = A2^T block (DRAM fp8)
                        diag_f8(5, b, a2t8b, A48[:, :, cs])
                        # D7 = diag(A3*A4)
                        diag_f8(6, b, A3T8[:, :, cs], A48[:, :, cs])
                        # D8 = diag(A4*A4) in bf16: lhsT = A4^T transpose chunk,
                        # rhs = A4 col-block streamed from DRAM.
                        dps = dpsp.tile([P, P], f32, tag="dg")
                        for kk in range(ko):
                            nc.tensor.matmul(
                                dps,
                                a4tb[:, kk, :],
                                a4cb[:, kk, :],
                                start=(kk == 0),
                                stop=(kk == ko - 1),
                            )
                        dt = evp.tile([P, P], f32, tag="dtmp")
                        nc.vector.tensor_tensor(dt, dps, identf, mult)
                        nc.vector.tensor_reduce(Dall[:, b, 7:8], dt, AX, add)  # D8
                        if b >= 2:
                            p5_t(b - 2)
                        if b >= 3:
                            p5_y(b - 3)

                    p5_t(nb - 2)
                    p5_t(nb - 1)
                    for no in (nb - 3, nb - 2, nb - 1):
                        p5_y(no)
                    nc.sync.dma_start(
                        Y_out.ap().rearrange("(no ni) c -> ni no c", ni=P), y_sb
                    )

    nc.compile()
    return nc


def _make_runner(nc):
    """Cached jitted SPMD executor (mirrors bass2jax.run_bass_via_pjrt)."""
    import jax
    from jax.experimental.shard_map import shard_map
    from jax.sharding import Mesh, PartitionSpec

    import concourse.mybir as mybir
    from concourse.bass2jax import (
        _bass_exec_p,
        install_neuronx_cc_hook,
        partition_id_tensor,
    )

    install_neuronx_cc_hook()
    partition_name = nc.partition_id_tensor.name if nc.partition_id_tensor else None

    in_names, out_names, out_avals, zero_outs = [], [], [], []
    for alloc in nc.m.functions[0].allocations:
        if not isinstance(alloc, mybir.MemoryLocationSet):
            continue
        name = alloc.memorylocations[0].name
        if alloc.kind == "ExternalInput":
            if name != partition_name:
                in_names.append(name)
        elif alloc.kind == "ExternalOutput":
            shape = tuple(alloc.tensor_shape)
            dtype = mybir.dt.np(alloc.dtype)
            out_names.append(name)
            out_avals.append(jax.core.ShapedArray(shape, dtype))
            zero_outs.append(np.zeros(shape, dtype))
    n_params = len(in_names)
    n_outs = len(out_avals)
    all_in_names = list(in_names) + list(out_names)
    if partition_name is not None:
        all_in_names.append(partition_name)

    def _body(*args):
        operands = list(args)
        if partition_name is not None:
            operands.append(partition_id_tensor())
        outs = _bass_exec_p.bind(
            *operands,
            out_avals=tuple(out_avals),
            in_names=tuple(all_in_names),
            out_names=tuple(out_names),
            lowering_input_output_aliases=(),
            sim_require_finite=True,
            sim_require_nnan=True,
            nc=nc,
        )
        return tuple(outs)

    devices = jax.devices()[:N_CORES]
    assert len(devices) == N_CORES, f"need {N_CORES} cores, got {len(devices)}"
    mesh = Mesh(np.asarray(devices), ("core",))
    in_specs = (PartitionSpec("core"),) * (n_params + n_outs)
    out_specs = (PartitionSpec("core"),) * n_outs
    sharded = jax.jit(
        shard_map(
            _body, mesh=mesh, in_specs=in_specs, out_specs=out_specs, check_rep=False
        ),
        donate_argnums=tuple(range(n_params, n_params + n_outs)),
        keep_unused=True,
    )
    return sharded, in_names, out_names, out_avals, zero_outs


def _make_hh(h: np.ndarray) -> np.ndarray:
    """hh rows 0..7 = h[1..8] with the fp8 storage scales folded back in;
    row 8 = h[0] (identity term)."""
    hh = np.zeros((16, C), np.float32)
    hh[0:NUM_TERMS] = h[1 : NUM_TERMS + 1]
    hh[2] /= S_A2                 # D3 PSUM carries S_A2
    hh[4] /= S_A3 * S_A2 * S_A2   # D5 = (S_A2*S_A3 A3) * (S_A2 A2)
    hh[5] /= S_A2 * S_A4          # D6 = (S_A2 A2^T) * (S_A4 A4)
    hh[6] /= S_A3 * S_A2 * S_A4   # D7 = (S_A2*S_A3 A3) * (S_A4 A4)
    hh[NUM_TERMS] = h[0]
    return hh


def _prep_inputs(A: np.ndarray, h: np.ndarray):
    A_bf = _bf16(A)  # [B, N, N] bf16
    return A_bf, _make_hh(h)


def kernel(A: np.ndarray, h: np.ndarray) -> np.ndarray:
    global _RUNNER, _NC
    import jax
    from jax.sharding import Mesh, NamedSharding, PartitionSpec

    A = np.ascontiguousarray(A, np.float32)
    h = np.ascontiguousarray(h, np.float32)
    if _RUNNER is None:
        _NC = _build(N)
        _RUNNER = _make_runner(_NC)
    sharded, in_names, out_names, out_avals, zero_outs = _RUNNER

    devices = jax.devices()[:N_CORES]
    mesh = Mesh(np.asarray(devices), ("core",))
    sh = NamedSharding(mesh, PartitionSpec("core"))

    # Pipeline the host bf16 cast with the per-shard H2D transfers.
    hh = _make_hh(h)
    shard_bufs = []
    for b in range(BATCH):
        shard_bufs.append(jax.device_put(_bf16(A[b]), devices[b]))
    A_dev = jax.make_array_from_single_device_arrays((BATCH * N, N), sh, shard_bufs)
    hh_dev = jax.make_array_from_single_device_arrays(
        (BATCH * 16, C),
        sh,
        [jax.device_put(hh, d) for d in devices],
    )
    per_name = {"A": A_dev, "hh": hh_dev}
    concat_in = [per_name[name] for name in in_names]
    concat_zeros = [
        np.zeros((BATCH * z.shape[0], *z.shape[1:]), z.dtype) for z in zero_outs
    ]
    outs = sharded(*concat_in, *concat_zeros)
    y = np.asarray(outs[out_names.index("Y")]).reshape(BATCH, N, C)
    return np.ascontiguousarray(y, np.float32)
